# revision 1
# baseline (speedup 1.0000x reference)
"""Single-head attention (QKV proj + softmax attention) for TRN2, 8 NeuronCores.

Problem: x [4, 2048, 1024] f32; Wq/Wk/Wv [1024, 1024]; bq/bk/bv [1024].
    q = x @ Wq.T + bq ; k = x @ Wk.T + bk ; v = x @ Wv.T + bv
    out = softmax(q k^T / sqrt(1024)) v            -> [4, 2048, 1024]

Sharding: 8 shards = (batch b, query-half h). Each core gets its batch's x^T
(for K/V), its query-slice x^T, and W^T — all transposed and rounded to the
fp32r grid on the host (layout prep, no FLOPs) — and computes its 1024 output
rows. No collectives; the host stitches the 8 outputs.

On-core pipeline (fp32r matmuls for projections/scores: fp32 with 12-bit
mantissa inputs, fp32 accumulation, 4x the PE rate of plain fp32; fp16 for
the attention-weighted sum):

  A:  V = (xkvT.T WvT + bv)      -> resident fp16 [skv, d]
      kT = (WkT.T xkvT) + bk     -> resident f32r [d, skv]
  B:  qT = (WqT.T xqT + bq)/sqrt(d) -> resident f32r [d, sq]
  C:  per 128-query tile: scores = qT.T kT -> softmax (max/exp/sum)
      -> probs fp16 -> PE transpose -> attn@V fp16 -> *1/l -> out
"""

import math
import os
import numpy as np

P = 128
NCH = 512  # psum free-dim chunk (one fp32 bank)

_cache = {}


def _build_program(D, SQ, SKV, n_cores, repeat=1):
    import concourse.bass as bass
    import concourse.tile as tile
    from concourse import bacc, mybir
    from concourse.masks import make_identity
    from contextlib import ExitStack

    f32 = mybir.dt.float32
    f32r = mybir.dt.float32r
    f16 = mybir.dt.float16
    Act = mybir.ActivationFunctionType
    AX = mybir.AxisListType

    dt_ = D // P          # d tiles
    sqt = SQ // P         # query tiles per core
    skt = SKV // P        # key/value tiles
    nck = min(NCH, SKV)
    ncq = min(NCH, SQ)
    ncd = min(NCH, D)
    nq = SQ // ncq
    nkv = SKV // nck
    nd = D // ncd
    scale = 1.0 / math.sqrt(D)
    TG = 4
    TGk = min(TG, skt)
    nckh = min(nck, SQ)   # A-phase chunk, must not cross the half boundary

    nc = bacc.Bacc("TRN2", target_bir_lowering=False, debug=False,
                   num_devices=n_cores)

    xkvt_d = nc.dram_tensor("xkvT", [D, SKV], f32r, kind="ExternalInput").ap()
    wqt_d = nc.dram_tensor("WqT", [D, D], f32r, kind="ExternalInput").ap()
    wkt_d = nc.dram_tensor("WkT", [D, D], f32r, kind="ExternalInput").ap()
    wvt_d = nc.dram_tensor("WvT", [D, D], f16, kind="ExternalInput").ap()
    bq_d = nc.dram_tensor("bq", [D], f32, kind="ExternalInput").ap()
    bk_d = nc.dram_tensor("bk", [D], f32, kind="ExternalInput").ap()
    bv_d = nc.dram_tensor("bv", [D], f32, kind="ExternalInput").ap()
    out_d = nc.dram_tensor("out", [SQ, D], f32, kind="ExternalOutput").ap()

    with tile.TileContext(nc, pool_alloc_mode="queue") as tc, ExitStack() as ctx:
        const = ctx.enter_context(tc.tile_pool(name="const", bufs=1))
        ident = const.tile([P, P], f32)
        make_identity(nc, ident[:])
        ident_h = const.tile([P, P], f16)
        nc.vector.tensor_copy(ident_h[:], ident[:])

        bq_raw = const.tile([P, dt_], f32)
        nc.sync.dma_start(bq_raw[:], bq_d.rearrange("(j p) -> p j", p=P))
        bqs = const.tile([P, dt_], f32)
        nc.vector.tensor_scalar_mul(bqs[:], bq_raw[:], scale)
        bkc = const.tile([P, dt_], f32)
        nc.sync.dma_start(bkc[:], bk_d.rearrange("(j p) -> p j", p=P))
        bvb = const.tile([P, D], f32)
        nc.gpsimd.dma_start(
            out=bvb[:],
            in_=bv_d.rearrange("(a d) -> a d", a=1).to_broadcast([P, D]))
        r_all = const.tile([P, sqt], f32)

        for _rep in range(repeat):
            kt_pool = tc.alloc_tile_pool(name="ktp", bufs=1)
            kT = [kt_pool.tile([P, SKV], f32r, name=f"kT{i}", tag=f"kT{i}")
                  for i in range(dt_)]
            v_pool = tc.alloc_tile_pool(name="vp", bufs=1)
            V = [v_pool.tile([P, D], f16, name=f"V{i}", tag=f"V{i}")
                 for i in range(skt)]

            # ============ scope A: V, kT ============
            # keys are stored with this core's query half FIRST (host-side
            # rotation; attention is permutation-invariant over keys), so
            # half A doubles as the query slice for scope B.
            xkva_pool = tc.alloc_tile_pool(name="xkvap", bufs=1)
            xkvA = [xkva_pool.tile([P, SQ], f32r, name=f"xkvA{i}",
                                   tag=f"xkvA{i}") for i in range(dt_)]
            xkvb_pool = tc.alloc_tile_pool(name="xkvbp", bufs=1)
            xkvB = [xkvb_pool.tile([P, SKV - SQ], f32r, name=f"xkvB{i}",
                                   tag=f"xkvB{i}") for i in range(dt_)]

            def xkv(k, c0, w):
                if c0 + w <= SQ:
                    return xkvA[k][:, c0:c0 + w]
                assert c0 >= SQ
                return xkvB[k][:, c0 - SQ:c0 - SQ + w]
            wva = tc.alloc_tile_pool(name="wva", bufs=1)
            psm = tc.alloc_tile_pool(name="psm", bufs=4, space="PSUM")
            wvT = [wva.tile([P, D], f16, name=f"wvT{i}", tag=f"wvT{i}")
                   for i in range(dt_)]
            xsh = [wva.tile([P, SQ], f16, name=f"xsh{i}", tag=f"xsh{i}")
                   for i in range(dt_)]
            for ch in range(SKV // nckh):
                for i in range(dt_):
                    nc.sync.dma_start(xkv(i, ch * nckh, nckh),
                                      xkvt_d[i * P:(i + 1) * P,
                                             ch * nckh:(ch + 1) * nckh])
                if ch < nd:
                    for i in range(dt_):
                        nc.sync.dma_start(wvT[i][:, ch * ncd:(ch + 1) * ncd],
                                          wvt_d[i * P:(i + 1) * P,
                                                ch * ncd:(ch + 1) * ncd])
            halves_m = [range(0, SQ // P), range(SQ // P, skt)]
            for hi, mrange in enumerate(halves_m):
                if len(mrange) == 0:
                    continue
                src = xkvA if hi == 0 else xkvB
                for k in range(dt_):
                    nc.scalar.activation(xsh[k][:, :len(mrange) * P],
                                         src[k][:, :len(mrange) * P], Act.Copy)
                for n in range(nd):
                    for m in mrange:
                        lm = m - mrange[0]
                        pv = psm.tile([P, ncd], f32, tag="pv")
                        for k in range(dt_):
                            nc.tensor.matmul(pv[:], xsh[k][:, lm * P:(lm + 1) * P],
                                             wvT[k][:, n * ncd:(n + 1) * ncd],
                                             start=(k == 0), stop=(k == dt_ - 1))
                        nc.vector.tensor_add(V[m][:, n * ncd:(n + 1) * ncd],
                                             pv[:], bvb[:, n * ncd:(n + 1) * ncd])
            wva.release()
            psm.release()

            wka = tc.alloc_tile_pool(name="wka", bufs=1)
            psk = tc.alloc_tile_pool(name="psk", bufs=4, space="PSUM")
            wkT = [wka.tile([P, D], f32r, name=f"wkT{i}", tag=f"wkT{i}")
                   for i in range(dt_)]
            for ch in range(nd):
                for i in range(dt_):
                    nc.sync.dma_start(wkT[i][:, ch * ncd:(ch + 1) * ncd],
                                      wkt_d[i * P:(i + 1) * P,
                                            ch * ncd:(ch + 1) * ncd])
            for m in range(dt_):
                for n in range(SKV // nckh):
                    pk = psk.tile([P, nckh], f32, tag="pk")
                    for k in range(dt_):
                        nc.tensor.matmul(pk[:], wkT[k][:, m * P:(m + 1) * P],
                                         xkv(k, n * nckh, nckh),
                                         start=(k == 0), stop=(k == dt_ - 1))
                    nc.scalar.activation(kT[m][:, n * nckh:(n + 1) * nckh],
                                         pk[:], Act.Identity,
                                         bias=bkc[:, m:m + 1])
            psk.release()
            wka.release()
            xkvb_pool.release()

            # ============ scope B: qT (reads query half of xkv) ============
            qt_pool = tc.alloc_tile_pool(name="qtp", bufs=1)
            qT = [qt_pool.tile([P, SQ], f32r, name=f"qT{i}", tag=f"qT{i}")
                  for i in range(dt_)]
            wqa = tc.alloc_tile_pool(name="wqa", bufs=1)
            psq = tc.alloc_tile_pool(name="psq", bufs=4, space="PSUM")
            wqT = [wqa.tile([P, D], f32r, name=f"wqT{i}", tag=f"wqT{i}")
                   for i in range(dt_)]
            for ch in range(nd):
                for i in range(dt_):
                    nc.sync.dma_start(wqT[i][:, ch * ncd:(ch + 1) * ncd],
                                      wqt_d[i * P:(i + 1) * P,
                                            ch * ncd:(ch + 1) * ncd])
            for n in range(nq):
                for m in range(dt_):
                    pq = psq.tile([P, ncq], f32, tag="pq")
                    for k in range(dt_):
                        nc.tensor.matmul(pq[:], wqT[k][:, m * P:(m + 1) * P],
                                         xkvA[k][:, n * ncq:(n + 1) * ncq],
                                         start=(k == 0), stop=(k == dt_ - 1))
                    nc.scalar.activation(qT[m][:, n * ncq:(n + 1) * ncq], pq[:],
                                         Act.Identity, bias=bqs[:, m:m + 1],
                                         scale=scale)
            psq.release()
            wqa.release()

            # ============ scope C: scores -> softmax -> attn@V -> out ======
            wc = tc.alloc_tile_pool(name="wc", bufs=2)
            pss = tc.alloc_tile_pool(name="pss", bufs=2, space="PSUM")
            pst = tc.alloc_tile_pool(name="pst", bufs=2, space="PSUM")
            pso = tc.alloc_tile_pool(name="pso", bufs=2, space="PSUM")
            for q in range(sqt):
                s = wc.tile([P, SKV], f32, tag="s")
                for n in range(nkv):
                    ps = pss.tile([P, nck], f32, tag="ps")
                    for k in range(dt_):
                        nc.tensor.matmul(ps[:], qT[k][:, q * P:(q + 1) * P],
                                         kT[k][:, n * nck:(n + 1) * nck],
                                         start=(k == 0), stop=(k == dt_ - 1))
                    nc.scalar.activation(s[:, n * nck:(n + 1) * nck], ps[:],
                                         Act.Copy)
                mneg = wc.tile([P, 1], f32, tag="mneg")
                nc.vector.tensor_reduce(mneg[:], s[:], axis=AX.X,
                                        op=mybir.AluOpType.max, negate=True)
                p_t = wc.tile([P, SKV], f16, tag="p_t")
                l_t = wc.tile([P, 1], f32, tag="l_t")
                nc.scalar.activation(p_t[:], s[:], Act.Exp, bias=mneg[:],
                                     accum_out=l_t[:])
                nc.vector.reciprocal(r_all[:, q:q + 1], l_t[:])
                strips = []
                for j in range(skt // TGk):
                    pt = pst.tile([P, TGk * P], f16, tag="pt")
                    for jj in range(TGk):
                        c = j * TGk + jj
                        nc.tensor.matmul(pt[:, jj * P:(jj + 1) * P],
                                         p_t[:, c * P:(c + 1) * P], ident_h[:],
                                         is_transpose=True,
                                         start=(jj == 0), stop=(jj == TGk - 1))
                    st = wc.tile([P, TGk * P], f16, tag=f"st{j}", bufs=2)
                    nc.vector.tensor_copy(st[:], pt[:])
                    strips.append(st)
                po = pso.tile([P, D], f32, tag="po")
                for c in range(skt):
                    for n2 in range(nd):
                        nc.tensor.matmul(po[:, n2 * ncd:(n2 + 1) * ncd],
                                         strips[c // TGk][:, (c % TGk) * P:
                                                          (c % TGk + 1) * P],
                                         V[c][:, n2 * ncd:(n2 + 1) * ncd],
                                         start=(c == 0), stop=(c == skt - 1))
                ot = wc.tile([P, D], f32, tag="ot")
                nc.vector.tensor_scalar_mul(ot[:], po[:], r_all[:, q:q + 1])
                nc.sync.dma_start(out_d[q * P:(q + 1) * P, :], ot[:])

            pso.release()
            pst.release()
            pss.release()
            wc.release()
            qt_pool.release()
            xkva_pool.release()
            v_pool.release()
            kt_pool.release()

    nc.compile()
    return nc


def get_program(D=1024, SQ=1024, SKV=2048, n_cores=8, repeat=1):
    key = (D, SQ, SKV, n_cores, repeat)
    if key not in _cache:
        _cache[key] = _build_program(D, SQ, SKV, n_cores, repeat)
    return _cache[key]


def _round_f32r(a):
    """Round-to-nearest onto the fp32r grid (keep top 11 mantissa bits)."""
    u = np.ascontiguousarray(a, dtype=np.float32).view(np.uint32)
    r = ((u + np.uint32(0x800)) & np.uint32(0xFFFFF000)).view(np.float32)
    return r


def kernel(x, Wq, bq, Wk, bk, Wv, bv):
    from concourse.bass_utils import run_bass_kernel_spmd

    x = np.asarray(x, dtype=np.float32)
    B, S, D = x.shape
    n_cores = 8
    halves = n_cores // B          # query-shards per batch
    SQ = S // halves

    nc = get_program(D=D, SQ=SQ, SKV=S, n_cores=n_cores)

    wqt = _round_f32r(np.asarray(Wq, dtype=np.float32).T)
    wkt = _round_f32r(np.asarray(Wk, dtype=np.float32).T)
    wvt = np.ascontiguousarray(np.asarray(Wv, dtype=np.float32).T.astype(np.float16))
    bq = np.asarray(bq, dtype=np.float32)
    bk = np.asarray(bk, dtype=np.float32)
    bv = np.asarray(bv, dtype=np.float32)

    xkvt = [_round_f32r(x[b].T) for b in range(B)]
    in_maps = []
    for c in range(n_cores):
        b, h = divmod(c, halves)
        xr = np.ascontiguousarray(
            np.roll(xkvt[b], -h * SQ, axis=1))  # this core's queries first
        in_maps.append({
            "xkvT": xr,
            "WqT": wqt, "WkT": wkt, "WvT": wvt,
            "bq": bq, "bk": bk, "bv": bv,
        })
    res = run_bass_kernel_spmd(nc, in_maps, list(range(n_cores)),
                               trace=bool(os.environ.get("ATTN_TRACE")))
    kernel.last_results = res
    out = np.stack([res.results[c]["out"] for c in range(n_cores)])
    return np.ascontiguousarray(
        out.reshape(B, halves, SQ, D).reshape(B, S, D).astype(np.float32))


kernel.last_results = None

